# revision 25
# baseline (speedup 1.0000x reference)
import sys
from contextlib import ExitStack

import numpy as np

sys.path.insert(0, "/opt/trn_rl_repo")

import concourse.bass as bass
import concourse.bacc as bacc
import concourse.tile as tile
from concourse import mybir
from concourse.bass_utils import run_bass_kernel_spmd

F32 = mybir.dt.float32
BF16 = mybir.dt.bfloat16
F32R = mybir.dt.float32r
AF = mybir.ActivationFunctionType
OP = mybir.AluOpType
EPS = 1e-5
B, C, H, W = 16, 192, 48, 48
L = H * W                       # 2304
E, N, DTR = 384, 16, 12
NCORES = 8
BLOC = B // NCORES              # 2
TCH = 384                       # time chunk (8 rows)
NCH = L // TCH                  # 6
RPC = TCH // W                  # 8 rows per chunk
NSEG = 8                        # states per scan group
NGRP = 2
HP = H + 6
WP = W + 6
DEBUG = False


def _snake_order(Hh, Ww):
    o, d = [], []
    i, j, jd = 0, 0, "right"
    while i < Hh:
        o.append(i * Ww + j)
        if jd == "right":
            if j < Ww - 1:
                j += 1; d.append(1)
            else:
                i += 1; d.append(4); jd = "left"
        else:
            if j > 0:
                j -= 1; d.append(2)
            else:
                i += 1; d.append(4); jd = "right"
    d = [0] + d[:-1]
    return np.array(o), np.argsort(np.array(o)), np.array(d)


def _v(t, off, dims):
    return bass.AP(tensor=t.tensor, offset=t.offset + off, ap=[t.ap[0]] + dims)


def _build(A_row):
    nc = bacc.Bacc("TRN2", target_bir_lowering=False)
    dt = F32

    x_in = nc.dram_tensor("x_loc", [BLOC, C, L], F32R, kind="ExternalInput")
    w1t = nc.dram_tensor("w1t", [C, E], F32R, kind="ExternalInput")
    wdtt = nc.dram_tensor("wdtt", [E, E], F32R, kind="ExternalInput")
    wbct = nc.dram_tensor("wbct", [E, 2 * N], F32R, kind="ExternalInput")
    w2t = nc.dram_tensor("w2t", [E, C], F32R, kind="ExternalInput")
    dir2 = nc.dram_tensor("dir2", [N, 2 * W], dt, kind="ExternalInput")
    dirf = nc.dram_tensor("dirf", [N, 1], dt, kind="ExternalInput")
    cb1 = nc.dram_tensor("cb1", [128, 3], dt, kind="ExternalInput")
    cbdw = nc.dram_tensor("cbdw", [128, 3], dt, kind="ExternalInput")
    cb2dt = nc.dram_tensor("cb2dt", [128, 3], dt, kind="ExternalInput")
    cdp = nc.dram_tensor("cdp", [128, 3], dt, kind="ExternalInput")
    clng = nc.dram_tensor("clng", [128, 3], dt, kind="ExternalInput")
    clnb = nc.dram_tensor("clnb", [128, 3], dt, kind="ExternalInput")
    wdiag = nc.dram_tensor("wdiag", [3, 128, 49 * 128], F32R, kind="ExternalInput")
    cb2 = nc.dram_tensor("cb2", [128, 2], dt, kind="ExternalInput")
    conesr = nc.dram_tensor("conesr", [1, 128], F32R, kind="ExternalInput")
    conesc = nc.dram_tensor("conesc", [128, 1], F32R, kind="ExternalInput")
    czero = nc.dram_tensor("czero", [1, 3 * WP + 3], F32R, kind="ExternalInput")
    bc_stage = nc.dram_tensor("bc_stage", [BLOC, 2 * N, L], BF16, kind="Internal")
    xc_stage = nc.dram_tensor("xc_stage", [BLOC, 128, 3, L], F32R, kind="Internal")
    out_d = nc.dram_tensor("out_loc", [BLOC, C, L], dt, kind="ExternalOutput")
    if DEBUG:
        ych_d = nc.dram_tensor("ych_d", [BLOC, 128, 3 * L], dt, kind="ExternalOutput")

    with ExitStack() as ctx:
        tc = ctx.enter_context(tile.TileContext(nc))
        const = ctx.enter_context(tc.tile_pool(name="const", bufs=1))
        pconv = ctx.enter_context(tc.tile_pool(name="pconv", bufs=1))
        pstream = ctx.enter_context(tc.tile_pool(name="pstream", bufs=2))
        pxc = ctx.enter_context(tc.tile_pool(name="pxc", bufs=2))
        pch = ctx.enter_context(tc.tile_pool(name="pch", bufs=2))
        pscr = ctx.enter_context(tc.tile_pool(name="pscr", bufs=1))
        pcat = ctx.enter_context(tc.tile_pool(name="pcat", bufs=2))
        pbc = ctx.enter_context(tc.tile_pool(name="pbc", bufs=2))
        pnn = ctx.enter_context(tc.tile_pool(name="pnn", bufs=2))
        pln = ctx.enter_context(tc.tile_pool(name="pln", bufs=1))
        ppsum = ctx.enter_context(tc.tile_pool(name="ppsum", bufs=4, space="PSUM"))
        ppsc = ctx.enter_context(tc.tile_pool(name="ppsc", bufs=1, space="PSUM"))
        pmm2 = ctx.enter_context(tc.tile_pool(name="pmm2", bufs=1, space="PSUM"))
        pps1 = ctx.enter_context(tc.tile_pool(name="pps1", bufs=1, space="PSUM"))

        w1_sb = const.tile([128, 2, E], F32R)
        nc.sync.dma_start(out=w1_sb[:, 0, :], in_=w1t[0:128, :])
        nc.sync.dma_start(out=w1_sb[0:64, 1, :], in_=w1t[128:192, :])
        wdt_sb = const.tile([128, 3, E], F32R)
        wbc_sb = const.tile([128, 3, 2 * N], F32R)
        w2_sb = const.tile([128, 3, C], F32R)
        for k in range(3):
            nc.sync.dma_start(out=wdt_sb[:, k, :], in_=wdtt[k * 128:(k + 1) * 128, :])
            nc.sync.dma_start(out=wbc_sb[:, k, :], in_=wbct[k * 128:(k + 1) * 128, :])
            nc.sync.dma_start(out=w2_sb[:, k, :], in_=w2t[k * 128:(k + 1) * 128, :])
        dir2_sb = const.tile([N, 2 * W], dt)
        nc.sync.dma_start(out=dir2_sb, in_=dir2[:, :])
        dirf_sb = const.tile([N, 1], dt)
        nc.sync.dma_start(out=dirf_sb, in_=dirf[:, :])
        cols = {}
        for nm, src in [("b1", cb1), ("bdw", cbdw), ("b2dt", cb2dt),
                        ("dp", cdp), ("lng", clng), ("lnb", clnb)]:
            t = const.tile([128, 3], dt, tag=nm)
            nc.sync.dma_start(out=t, in_=src[:, :])
            cols[nm] = t
        b2_sb = const.tile([128, 2], dt)
        nc.sync.dma_start(out=b2_sb, in_=cb2[:, :])
        ones_r = const.tile([1, 128], F32R)
        nc.sync.dma_start(out=ones_r, in_=conesr[:, :])
        ones_c = const.tile([128, 1], F32R)
        nc.sync.dma_start(out=ones_c, in_=conesc[:, :])
        ones_b = const.tile([128, 1], dt)
        nc.vector.memset(ones_b, 1.0)
        zero_c = const.tile([128, 1], dt)
        nc.vector.memset(zero_c, 0.0)
        eps_c = const.tile([128, 1], dt)
        nc.vector.memset(eps_c, EPS)
        carry = [[const.tile([128, NSEG], dt, tag=f"cy{et}{g}", name=f"cy{et}{g}")
                  for g in range(NGRP)] for et in range(3)]

        w1r = w1_sb

        for b in range(BLOC):
            # ================= Stage A: in-proj + conv + SiLU -> xc_stage ====
            for et in range(3):
                dg = pconv.tile([128, 49 * 128], F32R, tag="diag", name="dg")
                nc.sync.dma_start(out=dg, in_=wdiag[et, :, :])
                hp = pconv.tile([128, HP * WP], F32R, tag="hpad", name="hp")
                # zero only the pad regions (DMA broadcast; memset can't write f32r)
                zap = czero[:, :]
                nc.sync.dma_start(out=_v(hp, 0, [[1, 3 * WP + 3]]), in_=bass.AP(
                    tensor=zap.tensor, offset=0, ap=[[0, 128], [1, 3 * WP + 3]]))
                nc.sync.dma_start(out=_v(hp, 51 * WP - 3, [[1, 3 * WP + 3]]), in_=bass.AP(
                    tensor=zap.tensor, offset=0, ap=[[0, 128], [1, 3 * WP + 3]]))
                nc.sync.dma_start(out=_v(hp, 3 * WP + 51, [[WP, H - 1], [1, 6]]), in_=bass.AP(
                    tensor=zap.tensor, offset=0, ap=[[0, 128], [0, H - 1], [1, 6]]))
                for tt in range(NCH):
                    xch = pstream.tile([128, TCH], F32R, tag="xch", name="xch")
                    xch2 = pstream.tile([64, TCH], F32R, tag="xch2", name="xch2")
                    nc.sync.dma_start(out=xch, in_=x_in[b, 0:128, tt * TCH:(tt + 1) * TCH])
                    nc.sync.dma_start(out=xch2, in_=x_in[b, 128:192, tt * TCH:(tt + 1) * TCH])
                    ps = pmm2.tile([128, TCH], dt, tag="mm2", name="ps")
                    nc.tensor.matmul(ps, w1r[:, 0, et * 128:(et + 1) * 128],
                                     xch, start=True, stop=False)
                    nc.tensor.matmul(ps, w1r[0:64, 1, et * 128:(et + 1) * 128],
                                     xch2, start=False, stop=True)
                    dst = _v(hp, (3 + tt * RPC) * WP + 3, [[WP, RPC], [1, W]])
                    src = _v(ps, 0, [[W, RPC], [1, W]])
                    nc.scalar.activation(dst, src, AF.Identity,
                                         bias=cols["b1"][:, et:et + 1], scale=1.0)
                for half in range(3):
                    cps = [ppsc.tile([128, TCH], dt, tag=f"c{j}", name=f"cps{j}")
                           for j in range(2)]
                    for tap in range(49):
                        dy, dx = tap // 7, tap % 7
                        stat = _v(dg, tap * 128, [[1, 128]])
                        for j in range(2):
                            i = half * 2 + j
                            mov = _v(hp, (i * RPC + dy) * WP + dx,
                                     [[WP, RPC], [1, W]])
                            nc.tensor.matmul(cps[j], stat, mov,
                                             start=(tap == 0), stop=(tap == 48))
                    for j in range(2):
                        i = half * 2 + j
                        sg = pnn.tile([128, TCH], dt, tag="sg", name="sg")
                        nc.scalar.activation(sg, cps[j], AF.Sigmoid,
                                             bias=cols["bdw"][:, et:et + 1], scale=1.0)
                        xcw = pnn.tile([128, TCH], F32R, tag="xcw", name="xcw")
                        nc.vector.scalar_tensor_tensor(
                            xcw, cps[j], cols["bdw"][:, et:et + 1], sg,
                            op0=OP.add, op1=OP.mult)
                        nc.sync.dma_start(out=xc_stage[b, :, et, i * TCH:(i + 1) * TCH],
                                          in_=xcw)

            # ====== Stage B/C: fused chunk loop with BC lookahead ======
            for et in range(3):
                for g in range(NGRP):
                    nc.vector.memset(carry[et][g], 0.0)

            def load_xc(ch):
                xcc = pxc.tile([128, 3, TCH], F32R, tag="xcc", name="xcc")
                nc.sync.dma_start(out=xcc, in_=xc_stage[b, :, :, ch * TCH:(ch + 1) * TCH])
                return xcc

            def readback(ch):
                t0 = ch * TCH
                bcg = []
                ccg = []
                for g in range(NGRP):
                    bg = pbc.tile([128, NSEG, TCH], BF16, tag=f"bcat{g}", name=f"bg{g}")
                    cg = pbc.tile([128, NSEG, TCH], BF16, tag=f"ccat{g}", name=f"cg{g}")
                    off = b * 2 * N * L + g * NSEG * L + t0
                    nc.sync.dma_start(out=bg, in_=bass.AP(
                        tensor=bc_ap.tensor, offset=off, ap=[[0, 128], [L, NSEG], [1, TCH]]))
                    nc.sync.dma_start(out=cg, in_=bass.AP(
                        tensor=bc_ap.tensor, offset=off + N * L,
                        ap=[[0, 128], [L, NSEG], [1, TCH]]))
                    bcg.append(bg)
                    ccg.append(cg)
                return bcg, ccg

            def dtproj(ch, xcc):
                xccr = xcc
                d_ch = pch.tile([128, 3, TCH], dt, tag="dch", name="d_ch")
                for eo in range(3):
                    psd = ppsum.tile([128, TCH], dt, tag="mm", name="psd")
                    for k in range(3):
                        nc.tensor.matmul(psd, wdt_sb[:, k, eo * 128:(eo + 1) * 128],
                                         xccr[:, k, :], start=(k == 0), stop=(k == 2))
                    nc.scalar.activation(d_ch[:, eo, :], psd, AF.Exp,
                                         bias=cols["b2dt"][:, eo:eo + 1], scale=1.0)
                    nc.scalar.activation(d_ch[:, eo, :], d_ch[:, eo, :], AF.Ln,
                                         bias=ones_b, scale=1.0)
                u_ch = pch.tile([128, 3, TCH], dt, tag="uch", name="u_ch")
                for et in range(3):
                    for par in range(2):
                        so = et * TCH + par * W + (W - 1 if par else 0)
                        src = _v(xcc, so, [[2 * W, RPC // 2], [-1 if par else 1, W]])
                        dst = _v(u_ch, et * TCH + par * W, [[2 * W, RPC // 2], [1, W]])
                        nc.gpsimd.tensor_copy(dst, src)
                du16 = pch.tile([128, 3, TCH], BF16, tag="du16", name="du16")
                nc.vector.tensor_mul(du16, d_ch, u_ch)
                return d_ch, u_ch, du16

            def bcproj(ch, xcc):
                t0 = ch * TCH
                xccr = xcc
                psbt = ppsum.tile([128, TCH], dt, tag="mm", name="psbt")
                psct = ppsum.tile([128, TCH], dt, tag="mm", name="psct")
                psb = psbt[0:N, :]
                psc = psct[0:N, :]
                for k in range(3):
                    nc.tensor.matmul(psb, wbc_sb[:, k, 0:N],
                                     xccr[:, k, :], start=(k == 0), stop=(k == 2))
                    nc.tensor.matmul(psc, wbc_sb[:, k, N:2 * N],
                                     xccr[:, k, :], start=(k == 0), stop=(k == 2))
                beff16 = pnn.tile([N, TCH], BF16, tag="beff16", name="beff16")
                c16 = pnn.tile([N, TCH], BF16, tag="c16", name="c16")
                nc.vector.tensor_add(beff16, psb,
                                     _v(dir2_sb, 0, [[0, TCH // (2 * W)], [1, 2 * W]]))
                if ch == 0:
                    nc.vector.tensor_add(beff16[:, 0:1], beff16[:, 0:1], dirf_sb)
                nc.vector.tensor_copy(c16, psc)
                nc.sync.dma_start(out=bc_stage[b, 0:N, t0:t0 + TCH], in_=beff16)
                nc.sync.dma_start(out=bc_stage[b, N:2 * N, t0:t0 + TCH], in_=c16)

            bc_ap = bc_stage[0, :, :]
            xcc0 = load_xc(0)
            bcproj(0, xcc0)
            st = {0: dtproj(0, xcc0) + readback(0)}
            for ch in range(NCH):
                t0 = ch * TCH
                if ch + 1 < NCH:
                    xcc1 = load_xc(ch + 1)
                    bcproj(ch + 1, xcc1)
                    st[ch + 1] = dtproj(ch + 1, xcc1) + readback(ch + 1)
                d_ch, u_ch, du16, bcg, ccg = st.pop(ch)

                ych = pch.tile([128, 3, TCH], F32R, tag="ych", name="ych")
                for et in range(3):
                    yp = pln.tile([128, NGRP, TCH], dt, tag="yp", name="yp")
                    ac = [None, None]
                    for g in range(NGRP):
                        a_cat = pcat.tile([128, NSEG, TCH], dt, tag="acat", name="a_cat")
                        ac[g] = a_cat
                        for n in range(4):
                            nc.scalar.activation(a_cat[:, n, :], d_ch[:, et, :], AF.Exp,
                                                 bias=zero_c, scale=float(A_row[g * NSEG + n]))
                        base = a_cat if g == 0 else ac[0]
                        for n in range(4, NSEG):
                            eng = nc.vector if n % 2 == 0 else nc.gpsimd
                            eng.tensor_mul(a_cat[:, n, :], a_cat[:, n - 4, :],
                                           base[:, 3, :])
                        b_cat = pcat.tile([128, NSEG, TCH], BF16, tag="bcat_in", name="b_cat")
                        nc.gpsimd.tensor_tensor(
                            b_cat, _v(du16, et * TCH, [[0, NSEG], [1, TCH]]),
                            bcg[g], op=OP.mult)
                        cslice = carry[et][g]
                        inj = pln.tile([128, NSEG], dt, tag="inj", name="inj")
                        nc.vector.tensor_mul(inj, _v(a_cat, 0, [[TCH, NSEG]]), cslice)
                        nc.vector.tensor_tensor(_v(b_cat, 0, [[TCH, NSEG]]),
                                                _v(b_cat, 0, [[TCH, NSEG]]), inj, op=OP.add)
                        nc.vector.memset(_v(a_cat, 0, [[TCH, NSEG]]), 0.0)
                        nc.vector.tensor_tensor_scan(
                            _v(b_cat, 0, [[1, NSEG * TCH]]),
                            _v(a_cat, 0, [[1, NSEG * TCH]]),
                            _v(b_cat, 0, [[1, NSEG * TCH]]),
                            initial=0.0, op0=OP.mult, op1=OP.add)
                        nc.vector.tensor_copy(cslice, _v(b_cat, TCH - 1, [[TCH, NSEG]]))
                        nc.vector.tensor_mul(b_cat, b_cat, ccg[g])
                        nc.gpsimd.tensor_tensor(
                            _v(b_cat, 0, [[1, 4 * TCH]]), _v(b_cat, 0, [[1, 4 * TCH]]),
                            _v(b_cat, 4 * TCH, [[1, 4 * TCH]]), op=OP.add)
                        nc.gpsimd.tensor_tensor(
                            _v(b_cat, 0, [[1, 2 * TCH]]), _v(b_cat, 0, [[1, 2 * TCH]]),
                            _v(b_cat, 2 * TCH, [[1, 2 * TCH]]), op=OP.add)
                        nc.gpsimd.tensor_tensor(
                            yp[:, g, :], _v(b_cat, 0, [[1, TCH]]),
                            _v(b_cat, TCH, [[1, TCH]]), op=OP.add)
                    nc.vector.tensor_add(yp[:, 0, :], yp[:, 0, :], yp[:, 1, :])
                    for par in range(2):
                        do = et * TCH + par * W + (W - 1 if par else 0)
                        dst = _v(ych, do, [[2 * W, RPC // 2], [-1 if par else 1, W]])
                        src0 = _v(u_ch, et * TCH + par * W, [[2 * W, RPC // 2], [1, W]])
                        src1 = _v(yp, par * W, [[2 * W, RPC // 2], [1, W]])
                        nc.vector.scalar_tensor_tensor(
                            dst, src0, cols["dp"][:, et:et + 1], src1,
                            op0=OP.mult, op1=OP.add)
                if DEBUG:
                    for eo in range(3):
                        nc.sync.dma_start(out=ych_d[b, :, eo * L + t0:eo * L + t0 + TCH],
                                          in_=ych[:, eo, :])

                # ---- LayerNorm + out-proj ----
                ysq = pscr.tile([128, 3, TCH], F32R, tag="scr", name="ysq")
                nc.scalar.activation(ysq, ych, AF.Square, bias=zero_c, scale=1.0)
                s1t = pps1.tile([1, TCH], dt, tag="spair", name="s1t")
                s2t = pmm2.tile([128, TCH], dt, tag="mm2", name="s2t")
                s1c = s1t[0:1, :]
                s2c = s2t[0:1, :]
                ocr = ones_c
                for et in range(3):
                    nc.tensor.matmul(s1c, ocr, ych[:, et, :],
                                     start=(et == 0), stop=(et == 2))
                for et in range(3):
                    nc.tensor.matmul(s2c, ocr, ysq[:, et, :],
                                     start=(et == 0), stop=(et == 2))
                muc = pln.tile([1, TCH], F32R, tag="lnA", name="muc")
                nc.scalar.activation(muc, s1c, AF.Copy, scale=1.0 / E)
                vc = pln.tile([1, TCH], F32R, tag="lnB", name="vc")
                nc.scalar.activation(vc, s2c, AF.Copy, scale=1.0 / E)
                m2 = pln.tile([1, TCH], dt, tag="lnC", name="m2")
                nc.vector.tensor_mul(m2, muc, muc)
                nc.vector.tensor_sub(vc, vc, m2)
                nc.scalar.activation(vc, vc, AF.Identity, bias=eps_c[0:1, :], scale=1.0)
                r0 = pln.tile([1, TCH], dt, tag="lnC", name="r0")
                nc.scalar.activation(r0, vc, AF.Ln, bias=zero_c[0:1, :], scale=1.0)
                rsc = pln.tile([1, TCH], F32R, tag="lnB", name="rsc")
                nc.scalar.activation(rsc, r0, AF.Exp, bias=zero_c[0:1, :], scale=-0.5)
                pmu = ppsum.tile([128, TCH], dt, tag="mm", name="pmu")
                prs = ppsum.tile([128, TCH], dt, tag="mm", name="prs")
                nc.tensor.matmul(pmu, ones_r, muc,
                                 start=True, stop=True)
                nc.tensor.matmul(prs, ones_r, rsc,
                                 start=True, stop=True)
                z_ch = pscr.tile([128, 3, TCH], F32R, tag="scr", name="z_ch")
                for et in range(3):
                    nc.vector.tensor_sub(z_ch[:, et, :], ych[:, et, :], pmu)
                    nc.vector.tensor_mul(z_ch[:, et, :], z_ch[:, et, :], prs)
                    nc.scalar.activation(z_ch[:, et, :], z_ch[:, et, :], AF.Relu,
                                         bias=cols["lnb"][:, et:et + 1],
                                         scale=cols["lng"][:, et:et + 1])
                zcr = z_ch
                for mt in range(2):
                    mr = 128 if mt == 0 else 64
                    po = ppsum.tile([128, TCH], dt, tag="mm", name="po")
                    for k in range(3):
                        nc.tensor.matmul(po[0:mr, :],
                                         w2_sb[:, k, mt * 128:mt * 128 + mr],
                                         zcr[:, k, :], start=(k == 0), stop=(k == 2))
                    ob = pnn.tile([128, TCH], dt, tag="ob", name="ob")
                    nc.scalar.activation(ob[0:mr, :], po[0:mr, :], AF.Identity,
                                         bias=b2_sb[0:mr, mt:mt + 1], scale=1.0)
                    nc.sync.dma_start(out=out_d[b, mt * 128:mt * 128 + mr, t0:t0 + TCH],
                                      in_=ob[0:mr, :])
    nc.compile()
    return nc


def _prep(inputs):
    f = lambda k: np.asarray(inputs[k], dtype=np.float32)
    x = f("x").reshape(B, C, L)
    s1 = f("bn1_g") / np.sqrt(f("bn1_v") + EPS)
    W1 = f("w_in") * s1[:, None]
    b1 = (f("b_in") - f("bn1_m")) * s1 + f("bn1_b")
    Wdt = f("w_dt") @ f("w_xproj")[:DTR]
    bias2 = 2.0 * f("b_dt")
    Wbc = f("w_xproj")[DTR:DTR + 2 * N].copy()
    Wbc[N:] *= 4.0
    A = -np.exp(f("A_log"))
    A_row = A[0].copy()
    order, inv_order, dirs = _snake_order(H, W)
    assert np.array_equal(order, inv_order)
    dB = f("dir_Bs")
    dir2 = np.empty((N, 2 * W), np.float32)
    dir2[:, 0] = dB[4]
    dir2[:, 1:W] = dB[1][:, None].T.repeat(W - 1, 0).T
    dir2[:, W] = dB[4]
    dir2[:, W + 1:] = dB[2][:, None].T.repeat(W - 1, 0).T
    dirT = dB[dirs].T
    assert np.allclose(np.tile(dir2, (1, L // (2 * W)))[:, 1:], dirT[:, 1:])
    dirf = (dB[0] - dB[4]).astype(np.float32)[:, None]
    Dp4 = 4.0 * f("Dp")
    s2 = f("bn2_g") / np.sqrt(f("bn2_v") + EPS)
    W2 = f("w_out") * s2[:, None]
    b2 = (f("b_out") - f("bn2_m")) * s2 + f("bn2_b")
    wdw = f("w_dw").reshape(E, 49)
    wdiag = np.zeros((3, 128, 49, 128), np.float32)
    for et in range(3):
        for p in range(128):
            wdiag[et, p, :, p] = wdw[et * 128 + p]
    wdiag = wdiag.reshape(3, 128, 49 * 128)

    def cols3(v):
        return np.ascontiguousarray(v.reshape(3, 128).T)

    consts = {
        "w1t": np.ascontiguousarray(W1.T),
        "wdtt": np.ascontiguousarray(Wdt.T),
        "wbct": np.ascontiguousarray(Wbc.T),
        "w2t": np.ascontiguousarray(W2.T),
        "dir2": np.ascontiguousarray(dir2),
        "dirf": dirf,
        "cb1": cols3(b1), "cbdw": cols3(f("b_dw")),
        "cb2dt": cols3(bias2),
        "cdp": cols3(Dp4), "clng": cols3(f("ln_g")), "clnb": cols3(f("ln_b")),
        "wdiag": np.ascontiguousarray(wdiag),
        "cb2": np.ascontiguousarray(np.pad(b2, (0, 64)).reshape(2, 128).T),
        "conesr": np.ones((1, 128), np.float32),
        "conesc": np.ones((128, 1), np.float32),
        "czero": np.zeros((1, 3 * WP + 3), np.float32),
    }
    return consts, x, A_row


_CACHE = {}


def kernel(**inputs):
    consts, x, A_row = _prep(inputs)

    if "prog" not in _CACHE:
        _CACHE["prog"] = _build(A_row)
    nc = _CACHE["prog"]

    in_maps = []
    for c in range(NCORES):
        m = dict(consts)
        m["x_loc"] = np.ascontiguousarray(x[c * BLOC:(c + 1) * BLOC])
        in_maps.append(m)
    res = run_bass_kernel_spmd(nc, in_maps, core_ids=list(range(NCORES)))
    _CACHE["last_res"] = res
    outs = [res.results[c]["out_loc"] for c in range(NCORES)]
    return np.concatenate(outs, axis=0).reshape(B, C, H, W).astype(np.float32)


# revision 31
# speedup vs baseline: 1.3184x; 1.3184x over previous
import sys
from contextlib import ExitStack

import numpy as np

sys.path.insert(0, "/opt/trn_rl_repo")

import concourse.bass as bass
import concourse.bacc as bacc
import concourse.tile as tile
from concourse import mybir
from concourse.bass_utils import run_bass_kernel_spmd

F32 = mybir.dt.float32
BF16 = mybir.dt.bfloat16
F32R = mybir.dt.float32r
AF = mybir.ActivationFunctionType
OP = mybir.AluOpType
EPS = 1e-5
B, C, H, W = 16, 192, 48, 48
L = H * W                       # 2304
E, N, DTR = 384, 16, 12
NCORES = 8
BLOC = B // NCORES              # 2
TCH = 384                       # time chunk (8 rows)
NCH = L // TCH                  # 6
RPC = TCH // W                  # 8 rows per chunk
NSEG = 8                        # states per scan group
NGRP = 2
HP = H + 6
WP = W + 6
DEBUG = False


def _snake_order(Hh, Ww):
    o, d = [], []
    i, j, jd = 0, 0, "right"
    while i < Hh:
        o.append(i * Ww + j)
        if jd == "right":
            if j < Ww - 1:
                j += 1; d.append(1)
            else:
                i += 1; d.append(4); jd = "left"
        else:
            if j > 0:
                j -= 1; d.append(2)
            else:
                i += 1; d.append(4); jd = "right"
    d = [0] + d[:-1]
    return np.array(o), np.argsort(np.array(o)), np.array(d)


def _v(t, off, dims):
    return bass.AP(tensor=t.tensor, offset=t.offset + off, ap=[t.ap[0]] + dims)


def _build(A_row):
    nc = bacc.Bacc("TRN2", target_bir_lowering=False)
    dt = F32

    x_in = nc.dram_tensor("x_loc", [BLOC, C, L], F32R, kind="ExternalInput")
    w1t = nc.dram_tensor("w1t", [C, E], F32R, kind="ExternalInput")
    wdtt = nc.dram_tensor("wdtt", [E, E], F32R, kind="ExternalInput")
    wbct = nc.dram_tensor("wbct", [E, 2 * N], F32R, kind="ExternalInput")
    w2t = nc.dram_tensor("w2t", [E, C], F32R, kind="ExternalInput")
    dir2 = nc.dram_tensor("dir2", [N, 2 * W], dt, kind="ExternalInput")
    dirf = nc.dram_tensor("dirf", [N, 1], dt, kind="ExternalInput")
    cb1 = nc.dram_tensor("cb1", [128, 3], dt, kind="ExternalInput")
    cbdw = nc.dram_tensor("cbdw", [128, 3], dt, kind="ExternalInput")
    cb2dt = nc.dram_tensor("cb2dt", [128, 3], dt, kind="ExternalInput")
    cdp = nc.dram_tensor("cdp", [128, 3], dt, kind="ExternalInput")
    clng = nc.dram_tensor("clng", [128, 3], dt, kind="ExternalInput")
    clnb = nc.dram_tensor("clnb", [128, 3], dt, kind="ExternalInput")
    wdiag = nc.dram_tensor("wdiag", [3, 128, 49 * 128], F32R, kind="ExternalInput")
    cb2 = nc.dram_tensor("cb2", [128, 2], dt, kind="ExternalInput")
    conesr = nc.dram_tensor("conesr", [1, 128], F32R, kind="ExternalInput")
    conesc = nc.dram_tensor("conesc", [128, 1], F32R, kind="ExternalInput")
    czero = nc.dram_tensor("czero", [1, 3 * WP + 3], F32R, kind="ExternalInput")
    bc_stage = nc.dram_tensor("bc_stage", [BLOC, 2 * N, L], BF16, kind="Internal")
    xc_stage = nc.dram_tensor("xc_stage", [BLOC, 128, 3, L], F32R, kind="Internal")
    out_d = nc.dram_tensor("out_loc", [BLOC, C, L], dt, kind="ExternalOutput")
    if DEBUG:
        ych_d = nc.dram_tensor("ych_d", [BLOC, 128, 3 * L], dt, kind="ExternalOutput")

    with ExitStack() as ctx:
        tc = ctx.enter_context(tile.TileContext(nc))
        const = ctx.enter_context(tc.tile_pool(name="const", bufs=1))
        pconv = ctx.enter_context(tc.tile_pool(name="pconv", bufs=1))
        pstream = ctx.enter_context(tc.tile_pool(name="pstream", bufs=2))
        pxc = ctx.enter_context(tc.tile_pool(name="pxc", bufs=2))
        pch = ctx.enter_context(tc.tile_pool(name="pch", bufs=2))
        pscr = ctx.enter_context(tc.tile_pool(name="pscr", bufs=1))
        pcat = ctx.enter_context(tc.tile_pool(name="pcat", bufs=3))
        pbc = ctx.enter_context(tc.tile_pool(name="pbc", bufs=2))
        pbcc = ctx.enter_context(tc.tile_pool(name="pbcc", bufs=1))
        pnn = ctx.enter_context(tc.tile_pool(name="pnn", bufs=2))
        pln = ctx.enter_context(tc.tile_pool(name="pln", bufs=1))
        ppsum = ctx.enter_context(tc.tile_pool(name="ppsum", bufs=4, space="PSUM"))
        ppsc = ctx.enter_context(tc.tile_pool(name="ppsc", bufs=1, space="PSUM"))
        pmm2 = ctx.enter_context(tc.tile_pool(name="pmm2", bufs=1, space="PSUM"))
        pps1 = ctx.enter_context(tc.tile_pool(name="pps1", bufs=1, space="PSUM"))

        w1_sb = const.tile([128, 2, E], F32R)
        nc.sync.dma_start(out=w1_sb[:, 0, :], in_=w1t[0:128, :])
        nc.sync.dma_start(out=w1_sb[0:64, 1, :], in_=w1t[128:192, :])
        wdt_sb = const.tile([128, 3, E], F32R)
        wbc_sb = const.tile([128, 3, 2 * N], F32R)
        w2_sb = const.tile([128, 3, C], F32R)
        for k in range(3):
            nc.sync.dma_start(out=wdt_sb[:, k, :], in_=wdtt[k * 128:(k + 1) * 128, :])
            nc.sync.dma_start(out=wbc_sb[:, k, :], in_=wbct[k * 128:(k + 1) * 128, :])
            nc.sync.dma_start(out=w2_sb[:, k, :], in_=w2t[k * 128:(k + 1) * 128, :])
        dir2_sb = const.tile([N, 2 * W], dt)
        nc.sync.dma_start(out=dir2_sb, in_=dir2[:, :])
        dirf_sb = const.tile([N, 1], dt)
        nc.sync.dma_start(out=dirf_sb, in_=dirf[:, :])
        cols = {}
        for nm, src in [("b1", cb1), ("bdw", cbdw), ("b2dt", cb2dt),
                        ("dp", cdp), ("lng", clng), ("lnb", clnb)]:
            t = const.tile([128, 3], dt, tag=nm)
            nc.sync.dma_start(out=t, in_=src[:, :])
            cols[nm] = t
        b2_sb = const.tile([128, 2], dt)
        nc.sync.dma_start(out=b2_sb, in_=cb2[:, :])
        ones_r = const.tile([1, 128], F32R)
        nc.sync.dma_start(out=ones_r, in_=conesr[:, :])
        ones_c = const.tile([128, 1], F32R)
        nc.sync.dma_start(out=ones_c, in_=conesc[:, :])
        ones_b = const.tile([128, 1], dt)
        nc.vector.memset(ones_b, 1.0)
        zero_c = const.tile([128, 1], dt)
        nc.vector.memset(zero_c, 0.0)
        eps_c = const.tile([128, 1], dt)
        nc.vector.memset(eps_c, EPS)
        carry = [[const.tile([128, NSEG], dt, tag=f"cy{et}{g}", name=f"cy{et}{g}")
                  for g in range(NGRP)] for et in range(3)]

        w1r = w1_sb

        for b in range(BLOC):
            # ================= Stage A: in-proj + conv + SiLU -> xc_stage ====
            for et in range(3):
                dg = pconv.tile([128, 49 * 128], F32R, tag="diag", name="dg")
                nc.sync.dma_start(out=dg, in_=wdiag[et, :, :])
                hp = pconv.tile([128, HP * WP], F32R, tag="hpad", name="hp")
                # zero only the pad regions (DMA broadcast; memset can't write f32r)
                zap = czero[:, :]
                nc.sync.dma_start(out=_v(hp, 0, [[1, 3 * WP + 3]]), in_=bass.AP(
                    tensor=zap.tensor, offset=0, ap=[[0, 128], [1, 3 * WP + 3]]))
                nc.sync.dma_start(out=_v(hp, 51 * WP - 3, [[1, 3 * WP + 3]]), in_=bass.AP(
                    tensor=zap.tensor, offset=0, ap=[[0, 128], [1, 3 * WP + 3]]))
                nc.sync.dma_start(out=_v(hp, 3 * WP + 51, [[WP, H - 1], [1, 6]]), in_=bass.AP(
                    tensor=zap.tensor, offset=0, ap=[[0, 128], [0, H - 1], [1, 6]]))
                def inproj(tt):
                    xch = pstream.tile([128, TCH], F32R, tag="xch", name="xch")
                    xch2 = pstream.tile([64, TCH], F32R, tag="xch2", name="xch2")
                    nc.sync.dma_start(out=xch, in_=x_in[b, 0:128, tt * TCH:(tt + 1) * TCH])
                    nc.sync.dma_start(out=xch2, in_=x_in[b, 128:192, tt * TCH:(tt + 1) * TCH])
                    ps = pmm2.tile([128, TCH], dt, tag="mm2", name="ps")
                    nc.tensor.matmul(ps, w1r[:, 0, et * 128:(et + 1) * 128],
                                     xch, start=True, stop=False)
                    nc.tensor.matmul(ps, w1r[0:64, 1, et * 128:(et + 1) * 128],
                                     xch2, start=False, stop=True)
                    dst = _v(hp, (3 + tt * RPC) * WP + 3, [[WP, RPC], [1, W]])
                    psrc = _v(ps, 0, [[W, RPC], [1, W]])
                    nc.scalar.activation(dst, psrc, AF.Identity,
                                         bias=cols["b1"][:, et:et + 1], scale=1.0)
                pre = {0: (0, 1, 2), 1: (3, 4), 2: (5,)}
                for half in range(3):
                    for tt in pre[half]:
                        inproj(tt)
                    cps = [ppsc.tile([128, TCH], dt, tag=f"c{j}", name=f"cps{j}")
                           for j in range(2)]
                    for tap in range(49):
                        dy, dx = tap // 7, tap % 7
                        stat = _v(dg, tap * 128, [[1, 128]])
                        for j in range(2):
                            i = half * 2 + j
                            mov = _v(hp, (i * RPC + dy) * WP + dx,
                                     [[WP, RPC], [1, W]])
                            nc.tensor.matmul(cps[j], stat, mov,
                                             start=(tap == 0), stop=(tap == 48))
                    for j in range(2):
                        i = half * 2 + j
                        sg = pnn.tile([128, TCH], dt, tag="sg", name="sg")
                        nc.scalar.activation(sg, cps[j], AF.Sigmoid,
                                             bias=cols["bdw"][:, et:et + 1], scale=1.0)
                        xcw = pnn.tile([128, TCH], F32R, tag="xcw", name="xcw")
                        nc.vector.scalar_tensor_tensor(
                            xcw, cps[j], cols["bdw"][:, et:et + 1], sg,
                            op0=OP.add, op1=OP.mult)
                        nc.sync.dma_start(out=xc_stage[b, :, et, i * TCH:(i + 1) * TCH],
                                          in_=xcw)

            # ====== Stage B/C: fused chunk loop with BC lookahead ======
            for et in range(3):
                for g in range(NGRP):
                    nc.vector.memset(carry[et][g], 0.0)

            def load_xc(ch):
                xcc = pxc.tile([128, 3, TCH], F32R, tag="xcc", name="xcc")
                nc.sync.dma_start(out=xcc, in_=xc_stage[b, :, :, ch * TCH:(ch + 1) * TCH])
                return xcc

            def readback(ch):
                t0 = ch * TCH
                bcg = []
                ccg = []
                for g in range(NGRP):
                    bg = pbc.tile([128, NSEG, TCH], BF16, tag=f"bcat{g}", name=f"bg{g}")
                    cg = pbcc.tile([128, NSEG, TCH], BF16, tag=f"ccat{g}", name=f"cg{g}")
                    off = b * 2 * N * L + g * NSEG * L + t0
                    nc.sync.dma_start(out=bg, in_=bass.AP(
                        tensor=bc_ap.tensor, offset=off, ap=[[0, 128], [L, NSEG], [1, TCH]]))
                    nc.sync.dma_start(out=cg, in_=bass.AP(
                        tensor=bc_ap.tensor, offset=off + N * L,
                        ap=[[0, 128], [L, NSEG], [1, TCH]]))
                    bcg.append(bg)
                    ccg.append(cg)
                return bcg, ccg

            def dtproj(ch, xcc):
                xccr = xcc
                d_ch = pch.tile([128, 3, TCH], dt, tag="dch", name="d_ch")
                for eo in range(3):
                    psd = ppsum.tile([128, TCH], dt, tag="mm", name="psd")
                    for k in range(3):
                        nc.tensor.matmul(psd, wdt_sb[:, k, eo * 128:(eo + 1) * 128],
                                         xccr[:, k, :], start=(k == 0), stop=(k == 2))
                    nc.scalar.activation(d_ch[:, eo, :], psd, AF.Exp,
                                         bias=cols["b2dt"][:, eo:eo + 1], scale=1.0)
                for eo in range(3):
                    nc.scalar.activation(d_ch[:, eo, :], d_ch[:, eo, :], AF.Ln,
                                         bias=ones_b, scale=1.0)
                u_ch = pch.tile([128, 3, TCH], dt, tag="uch", name="u_ch")
                for et in range(3):
                    for par in range(2):
                        so = et * TCH + par * W + (W - 1 if par else 0)
                        src = _v(xcc, so, [[2 * W, RPC // 2], [-1 if par else 1, W]])
                        dst = _v(u_ch, et * TCH + par * W, [[2 * W, RPC // 2], [1, W]])
                        nc.gpsimd.tensor_copy(dst, src)
                du16 = pch.tile([128, 3, TCH], BF16, tag="du16", name="du16")
                nc.gpsimd.tensor_mul(du16, d_ch, u_ch)
                return d_ch, u_ch, du16

            def bcproj(ch, xcc):
                t0 = ch * TCH
                xccr = xcc
                psbt = ppsum.tile([128, TCH], dt, tag="mm", name="psbt")
                psct = ppsum.tile([128, TCH], dt, tag="mm", name="psct")
                psb = psbt[0:N, :]
                psc = psct[0:N, :]
                for k in range(3):
                    nc.tensor.matmul(psb, wbc_sb[:, k, 0:N],
                                     xccr[:, k, :], start=(k == 0), stop=(k == 2))
                    nc.tensor.matmul(psc, wbc_sb[:, k, N:2 * N],
                                     xccr[:, k, :], start=(k == 0), stop=(k == 2))
                beff16 = pnn.tile([N, TCH], BF16, tag="beff16", name="beff16")
                c16 = pnn.tile([N, TCH], BF16, tag="c16", name="c16")
                nc.vector.tensor_add(beff16, psb,
                                     _v(dir2_sb, 0, [[0, TCH // (2 * W)], [1, 2 * W]]))
                if ch == 0:
                    nc.vector.tensor_add(beff16[:, 0:1], beff16[:, 0:1], dirf_sb)
                nc.vector.tensor_copy(c16, psc)
                nc.sync.dma_start(out=bc_stage[b, 0:N, t0:t0 + TCH], in_=beff16)
                nc.sync.dma_start(out=bc_stage[b, N:2 * N, t0:t0 + TCH], in_=c16)

            bc_ap = bc_stage[0, :, :]
            xcc0 = load_xc(0)
            bcproj(0, xcc0)
            st = {0: dtproj(0, xcc0) + readback(0)}
            for ch in range(NCH):
                t0 = ch * TCH
                if ch + 1 < NCH:
                    xcc1 = load_xc(ch + 1)
                    bcproj(ch + 1, xcc1)
                    st[ch + 1] = dtproj(ch + 1, xcc1) + readback(ch + 1)
                d_ch, u_ch, du16, bcg, ccg = st.pop(ch)

                ych = pch.tile([128, 3, TCH], F32R, tag="ych", name="ych")
                for et in range(3):
                    yp = pln.tile([128, NGRP, TCH], BF16, tag="yp", name="yp")
                    ac = [None, None]
                    for g in range(NGRP):
                        a_cat = pcat.tile([128, NSEG, TCH], dt, tag="acat", name="a_cat")
                        ac[g] = a_cat
                        for n in range(4):
                            nc.scalar.activation(a_cat[:, n, :], d_ch[:, et, :], AF.Exp,
                                                 bias=zero_c, scale=float(A_row[g * NSEG + n]))
                        base = a_cat if g == 0 else ac[0]
                        for n in range(4, NSEG):
                            nc.gpsimd.tensor_mul(a_cat[:, n, :], a_cat[:, n - 4, :],
                                                 base[:, 3, :])
                        b_cat = pcat.tile([128, NSEG, TCH], BF16, tag="bcat_in", name="b_cat")
                        nc.gpsimd.tensor_tensor(
                            b_cat, _v(du16, et * TCH, [[0, NSEG], [1, TCH]]),
                            bcg[g], op=OP.mult)
                        cslice = carry[et][g]
                        inj = pln.tile([128, NSEG], dt, tag="inj", name="inj")
                        nc.vector.tensor_mul(inj, _v(a_cat, 0, [[TCH, NSEG]]), cslice)
                        nc.vector.tensor_tensor(_v(b_cat, 0, [[TCH, NSEG]]),
                                                _v(b_cat, 0, [[TCH, NSEG]]), inj, op=OP.add)
                        nc.vector.memset(_v(a_cat, 0, [[TCH, NSEG]]), 0.0)
                        nc.vector.tensor_tensor_scan(
                            _v(b_cat, 0, [[1, NSEG * TCH]]),
                            _v(a_cat, 0, [[1, NSEG * TCH]]),
                            _v(b_cat, 0, [[1, NSEG * TCH]]),
                            initial=0.0, op0=OP.mult, op1=OP.add)
                        nc.vector.tensor_copy(cslice, _v(b_cat, TCH - 1, [[TCH, NSEG]]))
                        nc.vector.tensor_mul(b_cat, b_cat, ccg[g])
                        nc.gpsimd.tensor_tensor(
                            _v(b_cat, 0, [[1, 4 * TCH]]), _v(b_cat, 0, [[1, 4 * TCH]]),
                            _v(b_cat, 4 * TCH, [[1, 4 * TCH]]), op=OP.add)
                        nc.gpsimd.tensor_tensor(
                            _v(b_cat, 0, [[1, 2 * TCH]]), _v(b_cat, 0, [[1, 2 * TCH]]),
                            _v(b_cat, 2 * TCH, [[1, 2 * TCH]]), op=OP.add)
                        nc.gpsimd.tensor_tensor(
                            yp[:, g, :], _v(b_cat, 0, [[1, TCH]]),
                            _v(b_cat, TCH, [[1, TCH]]), op=OP.add)
                    nc.gpsimd.tensor_add(yp[:, 0, :], yp[:, 0, :], yp[:, 1, :])
                    for par in range(2):
                        do = et * TCH + par * W + (W - 1 if par else 0)
                        dst = _v(ych, do, [[2 * W, RPC // 2], [-1 if par else 1, W]])
                        src0 = _v(u_ch, et * TCH + par * W, [[2 * W, RPC // 2], [1, W]])
                        src1 = _v(yp, par * W, [[2 * W, RPC // 2], [1, W]])
                        nc.vector.scalar_tensor_tensor(
                            dst, src0, cols["dp"][:, et:et + 1], src1,
                            op0=OP.mult, op1=OP.add)
                if DEBUG:
                    for eo in range(3):
                        nc.sync.dma_start(out=ych_d[b, :, eo * L + t0:eo * L + t0 + TCH],
                                          in_=ych[:, eo, :])

                # ---- LayerNorm + out-proj ----
                ysq = pscr.tile([128, 3, TCH], F32R, tag="scr", name="ysq")
                nc.scalar.activation(ysq, ych, AF.Square, bias=zero_c, scale=1.0)
                s1t = pps1.tile([1, TCH], dt, tag="spair", name="s1t")
                s1c = s1t[0:1, :]
                ocr = ones_c
                for et in range(3):
                    nc.tensor.matmul(s1c, ocr, ych[:, et, :],
                                     start=(et == 0), stop=(et == 2))
                muc = pln.tile([1, TCH], F32R, tag="lnA", name="muc")
                nc.scalar.activation(muc, s1c, AF.Copy, scale=1.0 / E)
                s2t = pps1.tile([1, TCH], dt, tag="spair", name="s2t")
                s2c = s2t[0:1, :]
                for et in range(3):
                    nc.tensor.matmul(s2c, ocr, ysq[:, et, :],
                                     start=(et == 0), stop=(et == 2))
                vc = pln.tile([1, TCH], F32R, tag="lnB", name="vc")
                nc.scalar.activation(vc, s2c, AF.Copy, scale=1.0 / E)
                m2 = pln.tile([1, TCH], dt, tag="lnC", name="m2")
                nc.gpsimd.tensor_mul(m2, muc, muc)
                nc.gpsimd.tensor_sub(vc, vc, m2)
                nc.scalar.activation(vc, vc, AF.Identity, bias=eps_c[0:1, :], scale=1.0)
                r0 = pln.tile([1, TCH], dt, tag="lnC", name="r0")
                nc.scalar.activation(r0, vc, AF.Ln, bias=zero_c[0:1, :], scale=1.0)
                rsc = pln.tile([1, TCH], F32R, tag="lnB", name="rsc")
                nc.scalar.activation(rsc, r0, AF.Exp, bias=zero_c[0:1, :], scale=-0.5)
                pmu = ppsum.tile([128, TCH], dt, tag="mm", name="pmu")
                prs = ppsum.tile([128, TCH], dt, tag="mm", name="prs")
                nc.tensor.matmul(pmu, ones_r, muc,
                                 start=True, stop=True)
                nc.tensor.matmul(prs, ones_r, rsc,
                                 start=True, stop=True)
                z_ch = pscr.tile([128, 3, TCH], F32R, tag="scr", name="z_ch")
                for et in range(3):
                    nc.vector.tensor_sub(z_ch[:, et, :], ych[:, et, :], pmu)
                    nc.vector.tensor_mul(z_ch[:, et, :], z_ch[:, et, :], prs)
                    nc.scalar.activation(z_ch[:, et, :], z_ch[:, et, :], AF.Relu,
                                         bias=cols["lnb"][:, et:et + 1],
                                         scale=cols["lng"][:, et:et + 1])
                zcr = z_ch
                for mt in range(2):
                    mr = 128 if mt == 0 else 64
                    po = ppsum.tile([128, TCH], dt, tag="mm", name="po")
                    for k in range(3):
                        nc.tensor.matmul(po[0:mr, :],
                                         w2_sb[:, k, mt * 128:mt * 128 + mr],
                                         zcr[:, k, :], start=(k == 0), stop=(k == 2))
                    ob = pnn.tile([128, TCH], dt, tag="ob", name="ob")
                    nc.scalar.activation(ob[0:mr, :], po[0:mr, :], AF.Identity,
                                         bias=b2_sb[0:mr, mt:mt + 1], scale=1.0)
                    nc.sync.dma_start(out=out_d[b, mt * 128:mt * 128 + mr, t0:t0 + TCH],
                                      in_=ob[0:mr, :])
    nc.compile()
    return nc


def _prep(inputs):
    f = lambda k: np.asarray(inputs[k], dtype=np.float32)
    x = f("x").reshape(B, C, L)
    s1 = f("bn1_g") / np.sqrt(f("bn1_v") + EPS)
    W1 = f("w_in") * s1[:, None]
    b1 = (f("b_in") - f("bn1_m")) * s1 + f("bn1_b")
    Wdt = f("w_dt") @ f("w_xproj")[:DTR]
    bias2 = 2.0 * f("b_dt")
    Wbc = f("w_xproj")[DTR:DTR + 2 * N].copy()
    Wbc[N:] *= 4.0
    A = -np.exp(f("A_log"))
    A_row = A[0].copy()
    order, inv_order, dirs = _snake_order(H, W)
    assert np.array_equal(order, inv_order)
    dB = f("dir_Bs")
    dir2 = np.empty((N, 2 * W), np.float32)
    dir2[:, 0] = dB[4]
    dir2[:, 1:W] = dB[1][:, None].T.repeat(W - 1, 0).T
    dir2[:, W] = dB[4]
    dir2[:, W + 1:] = dB[2][:, None].T.repeat(W - 1, 0).T
    dirT = dB[dirs].T
    assert np.allclose(np.tile(dir2, (1, L // (2 * W)))[:, 1:], dirT[:, 1:])
    dirf = (dB[0] - dB[4]).astype(np.float32)[:, None]
    Dp4 = 4.0 * f("Dp")
    s2 = f("bn2_g") / np.sqrt(f("bn2_v") + EPS)
    W2 = f("w_out") * s2[:, None]
    b2 = (f("b_out") - f("bn2_m")) * s2 + f("bn2_b")
    wdw = f("w_dw").reshape(E, 49)
    wdiag = np.zeros((3, 128, 49, 128), np.float32)
    for et in range(3):
        for p in range(128):
            wdiag[et, p, :, p] = wdw[et * 128 + p]
    wdiag = wdiag.reshape(3, 128, 49 * 128)

    def cols3(v):
        return np.ascontiguousarray(v.reshape(3, 128).T)

    consts = {
        "w1t": np.ascontiguousarray(W1.T),
        "wdtt": np.ascontiguousarray(Wdt.T),
        "wbct": np.ascontiguousarray(Wbc.T),
        "w2t": np.ascontiguousarray(W2.T),
        "dir2": np.ascontiguousarray(dir2),
        "dirf": dirf,
        "cb1": cols3(b1), "cbdw": cols3(f("b_dw")),
        "cb2dt": cols3(bias2),
        "cdp": cols3(Dp4), "clng": cols3(f("ln_g")), "clnb": cols3(f("ln_b")),
        "wdiag": np.ascontiguousarray(wdiag),
        "cb2": np.ascontiguousarray(np.pad(b2, (0, 64)).reshape(2, 128).T),
        "conesr": np.ones((1, 128), np.float32),
        "conesc": np.ones((128, 1), np.float32),
        "czero": np.zeros((1, 3 * WP + 3), np.float32),
    }
    return consts, x, A_row


_CACHE = {}


def kernel(**inputs):
    consts, x, A_row = _prep(inputs)

    if "prog" not in _CACHE:
        _CACHE["prog"] = _build(A_row)
    nc = _CACHE["prog"]

    in_maps = []
    for c in range(NCORES):
        m = dict(consts)
        m["x_loc"] = np.ascontiguousarray(x[c * BLOC:(c + 1) * BLOC])
        in_maps.append(m)
    res = run_bass_kernel_spmd(nc, in_maps, core_ids=list(range(NCORES)))
    _CACHE["last_res"] = res
    outs = [res.results[c]["out_loc"] for c in range(NCORES)]
    return np.concatenate(outs, axis=0).reshape(B, C, H, W).astype(np.float32)
